# revision 14
# baseline (speedup 1.0000x reference)
"""Causal self-attention Trainium2 kernel.

B=2, T=2048, C=1024, H=16, D=64, 8 NeuronCores.
Sharding: core i handles batch b=i//4 and heads [4*(i%4), 4*(i%4)+4).
Host transposes x[b] -> xT, slices w_qkv/w_proj per core, and sums the 4
per-batch partial output projections at the end.

Structure: a single software-pipelined stream interleaved at 128-column
chunk granularity, so the PE never idles long enough for the HAM clock
gate to re-throttle it to 1.2 GHz:

  QKV(0); for nb: attn(*, nb) interleaved with {QKV(nb+1), norm+proj(nb-1)}

The QKV/projection matmuls (no ACT dependence) fill the PE while the
softmax exp stream (ACT-bound at ~1 col/ns vs the PE's 0.83 col/ns for
S+AV) catches up.

q/k/v and the exp'd scores P are bf16 (same 1 cyc/row PE rate as f32r,
half the SBUF/DMA traffic); accumulations all happen in fp32 PSUM.
Scores are computed transposed (S^T[j,i]) so exp/mask are free-dim ops
and P^T feeds the attention*V matmul as the moving operand. A ones
column appended to V yields the softmax denominator for free; its
reciprocal is computed on the DVE (keeps ACT exp-only: one table load)
and broadcast over partitions with a K=1 ones-matmul.
"""

import numpy as np
from contextlib import ExitStack

import concourse.bacc as bacc
import concourse.mybir as mybir
import concourse.tile as tile
from concourse.bass_utils import run_bass_kernel_spmd


B, T, C = 2, 2048, 1024
N_HEAD = 16
D = C // N_HEAD  # 64
N_CORES = 8
HPC = 4  # heads per core
TB = T // 512  # 4 i-blocks of 512
NJ = T // 128  # 16 j-chunks of 128

F32 = mybir.dt.float32
F32R = mybir.dt.float32r
BF16 = mybir.dt.bfloat16

_compiled = None


def _build_mask():
    """M[j, x] = 1.0 iff (x - 384) >= j, shape [128, 896].

    The [128,128] block M[:, 384:512] is the lower-triangular causal mask
    applied to the first 128 trimmed columns of each diagonal chunk.
    """
    j = np.arange(128)[:, None]
    x = np.arange(896)[None, :]
    return ((x - 384) >= j).astype(np.float32)


def _build_nc():
    nc = bacc.Bacc("TRN2", target_bir_lowering=False, debug=False,
                   num_devices=N_CORES)

    xt_t = nc.dram_tensor("xt", [C, T], BF16, kind="ExternalInput")
    wqk_t = nc.dram_tensor("wqk", [C, 8 * D], BF16, kind="ExternalInput")
    wv_t = nc.dram_tensor("wv", [C, 4 * D], BF16, kind="ExternalInput")
    wp_t = nc.dram_tensor("wp", [4 * D, C], F32R, kind="ExternalInput")
    mask_t = nc.dram_tensor("mask", [128, 896], BF16, kind="ExternalInput")
    ones_t = nc.dram_tensor("ones", [1, 64], BF16, kind="ExternalInput")
    out_t = nc.dram_tensor("out", [T, C], F32, kind="ExternalOutput")

    Exp = mybir.ActivationFunctionType.Exp

    with tile.TileContext(nc) as tc, ExitStack() as ctx:
        sb = ctx.enter_context(tc.tile_pool(name="sb", bufs=1))
        wk = ctx.enter_context(tc.tile_pool(name="wk", bufs=1))
        ps = ctx.enter_context(tc.tile_pool(name="ps", bufs=1, space="PSUM"))

        # ---- persistent SBUF ----
        mask_s = sb.tile([128, 896], BF16, tag="mask")
        ones_s = sb.tile([1, 64], BF16, tag="ones")
        wp_s = [sb.tile([128, C], F32R, tag=f"wp{p}", name=f"wp{p}")
                for p in range(2)]
        qT = [sb.tile([128, T], BF16, tag=f"qT{p}", name=f"qT{p}")
              for p in range(2)]
        # kT per head, full 128 partitions: the head's own 64 rows hold k,
        # the complementary 64 rows are zero so S^T matmuls run K=128
        kT = [sb.tile([128, T], BF16, tag=f"kT{h}", name=f"kT{h}")
              for h in range(HPC)]
        # v per head: [128 t-part, 128*NJ]; per 128-chunk: cols 0-63 = v,
        # col 64 = ones (softmax denominator), cols 65-127 = ones (padding
        # so AV matmuls run M=128 full-geometry; output rows 65-127 unused)
        v_s = [sb.tile([128, 128 * NJ], BF16, tag=f"v{h}", name=f"v{h}")
               for h in range(HPC)]
        yT = [sb.tile([128, T], F32R, tag=f"yT{p}", name=f"yT{p}")
              for p in range(2)]
        ytmp = sb.tile([64, 4096], F32R, tag="ytmp")
        wqk_s = [sb.tile([128, 8 * D], BF16, tag=f"wqk{kc}",
                         name=f"wqk{kc}") for kc in range(8)]
        wv_s = [sb.tile([128, 4 * D], BF16, tag=f"wv{kc}", name=f"wv{kc}")
                for kc in range(8)]
        xt_s = {(kc, nb): sb.tile([128, 512], BF16, tag=f"xt{kc}_{nb}",
                                  name=f"xt{kc}_{nb}")
                for nb in range(TB) for kc in range(8)}

        # ---- DMA priority order ----
        # First q/k weights + nb=0 activations (unblocks the matmul stream
        # ~2us in), then wv + mask, then deferred background loads.
        for kc in range(8):
            nc.sync.dma_start(wqk_s[kc][:],
                              wqk_t.ap()[128 * kc:128 * (kc + 1), :])
            nc.sync.dma_start(xt_s[(kc, 0)][:],
                              xt_t.ap()[128 * kc:128 * (kc + 1), 0:512])
        for kc in range(8):
            nc.sync.dma_start(wv_s[kc][:],
                              wv_t.ap()[128 * kc:128 * (kc + 1), :])
        nc.sync.dma_start(mask_s[:], mask_t.ap()[:])
        nc.sync.dma_start(ones_s[:], ones_t.ap()[:])

        # zero/ones init without DMA traffic
        for h in range(HPC):
            po = 64 * (h % 2)
            nc.gpsimd.memset(kT[h][64 - po:128 - po, :], 0.0)
            nc.gpsimd.memset(v_s[h][:], 1.0)

        with tc.tile_wait_until(0.008):
            for kc in range(8):
                nc.scalar.dma_start(xt_s[(kc, 1)][:],
                                    xt_t.ap()[128 * kc:128 * (kc + 1),
                                              512:1024])
            for p in range(2):
                nc.scalar.dma_start(wp_s[p][:],
                                    wp_t.ap()[128 * p:128 * (p + 1), :])
        with tc.tile_wait_until(0.018):
            for kc in range(8):
                nc.scalar.dma_start(xt_s[(kc, 2)][:],
                                    xt_t.ap()[128 * kc:128 * (kc + 1),
                                              1024:1536])
        with tc.tile_wait_until(0.028):
            for kc in range(8):
                nc.scalar.dma_start(xt_s[(kc, 3)][:],
                                    xt_t.ap()[128 * kc:128 * (kc + 1),
                                              1536:2048])

        # ---- QKV units (emitted directly for nb=0, as fillers after) ----
        def qkv_fillers(nb):
            """List of callables; each emits ~4 matmuls of QKV for nb."""
            fills = []

            def qk_unit(mc, half):
                def f():
                    if half == 0:
                        p = ps.tile([128, 512], F32, tag="qk", bufs=2,
                                    name="qk")
                        qk_unit.p = p
                        for kc in range(4):
                            nc.tensor.matmul(
                                p[:], wqk_s[kc][:, 128 * mc:128 * (mc + 1)],
                                xt_s[(kc, nb)][:],
                                start=(kc == 0), stop=False)
                    else:
                        p = qk_unit.p
                        for kc in range(4, 8):
                            nc.tensor.matmul(
                                p[:], wqk_s[kc][:, 128 * mc:128 * (mc + 1)],
                                xt_s[(kc, nb)][:],
                                start=False, stop=(kc == 7))
                        if mc < 2:
                            nc.vector.tensor_copy(
                                qT[mc][:, 512 * nb:512 * (nb + 1)], p[:])
                        else:
                            for s in range(2):
                                h = 2 * (mc - 2) + s
                                nc.vector.tensor_copy(
                                    kT[h][64 * s:64 * (s + 1),
                                          512 * nb:512 * (nb + 1)],
                                    p[64 * s:64 * (s + 1), :])
                return f

            def v_unit(tq, half):
                def f():
                    if half == 0:
                        p = ps.tile([128, 512], F32, tag="qk", bufs=2,
                                    name="qk")
                        v_unit.p = p
                        for kc in range(4):
                            nc.tensor.matmul(
                                p[:, 0:256],
                                xt_s[(kc, nb)][:, 128 * tq:128 * (tq + 1)],
                                wv_s[kc][:], start=(kc == 0), stop=False)
                    else:
                        p = v_unit.p
                        tci = 4 * nb + tq
                        for kc in range(4, 8):
                            nc.tensor.matmul(
                                p[:, 0:256],
                                xt_s[(kc, nb)][:, 128 * tq:128 * (tq + 1)],
                                wv_s[kc][:], start=False, stop=(kc == 7))
                        for h in range(HPC):
                            nc.vector.tensor_copy(
                                v_s[h][:, 128 * tci:128 * tci + 64],
                                p[:, 64 * h:64 * (h + 1)])
                return f

            # k first (attention depends on it first), then q, then v
            for mc in (2, 3, 0, 1):
                for half in range(2):
                    fills.append(qk_unit(mc, half))
            for tq in range(4):
                for half in range(2):
                    fills.append(v_unit(tq, half))
            return fills

        # ---- norm + proj fillers for a completed i-block ----
        recs = {}  # (h, ib) -> rec tile

        def norm_proj_fillers(ib):
            fills = []

            def norm_unit(h):
                def f():
                    prf = ps.tile([64, 512], F32, tag="bcast", bufs=1,
                                  name="prf")
                    nc.tensor.matmul(prf[:], ones_s[:],
                                     recs.pop((h, ib))[:],
                                     start=True, stop=True)
                    if h % 2 == 0:
                        nc.vector.tensor_mul(
                            yT[h // 2][0:64, 512 * ib:512 * (ib + 1)],
                            yT[h // 2][0:64, 512 * ib:512 * (ib + 1)],
                            prf[:])
                    else:
                        oidx = 4 * (h // 2) + ib
                        sl = ytmp[:, 512 * oidx:512 * (oidx + 1)]
                        nc.vector.tensor_mul(sl, sl, prf[:])
                        nc.scalar.dma_start(
                            yT[h // 2][64:128, 512 * ib:512 * (ib + 1)], sl)
                return f

            def proj_unit(tb, n):
                def f():
                    p = ps.tile([128, 512], F32, tag="mm", bufs=3,
                                name="mm")
                    for pp in range(2):
                        nc.tensor.matmul(
                            p[:], yT[pp][:, 128 * tb:128 * (tb + 1)],
                            wp_s[pp][:, 512 * n:512 * (n + 1)],
                            start=(pp == 0), stop=(pp == 1))
                    o = wk.tile([128, 512], F32, tag="o", bufs=4, name="o")
                    nc.vector.tensor_copy(o[:], p[:])
                    nc.sync.dma_start(
                        out_t.ap()[128 * tb:128 * (tb + 1),
                                   512 * n:512 * (n + 1)], o[:])
                return f

            for h in range(HPC):
                fills.append(norm_unit(h))
            for tb in range(4 * ib, 4 * ib + 4):
                for n in range(2):
                    fills.append(proj_unit(tb, n))
            return fills

        # ---- attention segment for i-block nb, with fillers woven in ----
        def attn_segment(nb, fills):
            jhi = 4 * nb + 3
            n_chunks = HPC * (jhi + 1)
            done_chunks = 0
            done_fills = 0

            def weave():
                nonlocal done_fills
                target = len(fills) * done_chunks // n_chunks
                while done_fills < target:
                    fills[done_fills]()
                    done_fills += 1

            for h in range(HPC):
                qTt = qT[h // 2]
                py = ps.tile([128, 512], F32, tag="avy", bufs=2, name="avy")
                avq = []

                def emit_av(ent, py=py, h=h, jhi=jhi):
                    jc, pt, off, w = ent
                    nc.tensor.matmul(
                        py[:, off:512],
                        v_s[h][:, 128 * jc:128 * (jc + 1)],
                        pt[:, 0:w], start=(jc == 0), stop=(jc == jhi))

                for jc in range(jhi + 1):
                    r = jc - 4 * nb
                    off = 128 * r if r > 0 else 0
                    w = 512 - off
                    p_s = ps.tile([128, 512], F32, tag="mm", bufs=3,
                                  name="mm")
                    nc.tensor.matmul(
                        p_s[:, 0:w],
                        kT[h][:, 128 * jc:128 * (jc + 1)],
                        qTt[:, 512 * nb + off:512 * (nb + 1)],
                        start=True, stop=True)
                    pt = wk.tile([128, 512], BF16, tag="pt", bufs=6,
                                 name="pt")
                    nc.scalar.activation(pt[:, 0:w], p_s[:, 0:w], Exp,
                                         scale=0.125)
                    if r >= 0:
                        # triangular sub-block = first 128 trimmed cols
                        nc.gpsimd.tensor_mul(
                            pt[:, 0:128], pt[:, 0:128],
                            mask_s[:, 384:512])
                    avq.append((jc, pt, off, w))
                    if len(avq) > 2:
                        emit_av(avq.pop(0))
                    done_chunks += 1
                    weave()
                while avq:
                    emit_av(avq.pop(0))
                # epilogue: denominator reciprocal. The custom-DVE recip
                # requires its input at partition 0, so stage the PSUM row
                # through SBUF and a partition-moving DMA first.
                dtmp = wk.tile([65, 512], F32, tag="dtmp", bufs=2,
                               name="dtmp")
                nc.vector.tensor_copy(dtmp[64:65, :], py[64:65, :])
                den0 = wk.tile([1, 512], F32, tag="den0", bufs=4,
                               name="den0")
                nc.scalar.dma_start(den0[:], dtmp[64:65, :])
                rec_f = wk.tile([1, 512], F32, tag="recf", bufs=2,
                                name="rec_f")
                nc.vector.reciprocal_approx_fast(rec_f[:], den0[:])
                rec = wk.tile([1, 512], BF16, tag="rec", bufs=8, name="rec")
                nc.vector.tensor_copy(rec[:], rec_f[:])
                recs[(h, nb)] = rec
                if h % 2 == 0:
                    nc.vector.tensor_copy(
                        yT[h // 2][0:64, 512 * nb:512 * (nb + 1)],
                        py[0:64, :])
                else:
                    oidx = 4 * (h // 2) + nb
                    nc.vector.tensor_copy(
                        ytmp[:, 512 * oidx:512 * (oidx + 1)], py[0:64, :])
            # drain remaining fillers
            while done_fills < len(fills):
                fills[done_fills]()
                done_fills += 1

        # ---- emit the pipeline ----
        for f in qkv_fillers(0):
            f()
        for nb in range(TB):
            fills = []
            qf = qkv_fillers(nb + 1) if nb + 1 < TB else []
            npf = norm_proj_fillers(nb - 1) if nb >= 1 else []
            # round-robin merge; norm/proj first (its data is ready)
            i = j = 0
            while i < len(npf) or j < len(qf):
                if i < len(npf):
                    fills.append(npf[i])
                    i += 1
                if j < len(qf):
                    fills.append(qf[j])
                    j += 1
            attn_segment(nb, fills)
        for f in norm_proj_fillers(TB - 1):
            f()

    nc.compile()
    return nc


def _get_compiled():
    global _compiled
    if _compiled is None:
        _compiled = _build_nc()
    return _compiled


def _in_maps(x, w_qkv, w_proj):
    import ml_dtypes

    bf16 = ml_dtypes.bfloat16
    x = np.asarray(x, dtype=np.float32)
    w_qkv = np.asarray(w_qkv, dtype=np.float32)
    w_proj = np.asarray(w_proj, dtype=np.float32)
    mask = _build_mask().astype(bf16)
    maps = []
    for core in range(N_CORES):
        b = core // 4
        h0 = 4 * (core % 4)
        heads = range(h0, h0 + HPC)
        xt = np.ascontiguousarray(x[b].T).astype(bf16)  # [C, T]
        wqk = np.concatenate(
            [w_qkv[:, 64 * h:64 * (h + 1)] for h in heads]
            + [w_qkv[:, C + 64 * h:C + 64 * (h + 1)] for h in heads],
            axis=1).astype(bf16)
        wv = np.concatenate(
            [w_qkv[:, 2 * C + 64 * h:2 * C + 64 * (h + 1)] for h in heads],
            axis=1).astype(bf16)
        wp = np.concatenate(
            [w_proj[64 * h:64 * (h + 1), :] for h in heads], axis=0)
        maps.append({
            "xt": np.ascontiguousarray(xt),
            "wqk": np.ascontiguousarray(wqk),
            "wv": np.ascontiguousarray(wv),
            "wp": np.ascontiguousarray(wp),
            "mask": mask,
            "ones": np.ones((1, 64), dtype=bf16),
        })
    return maps


def _combine(results, b_proj):
    out = np.zeros((B, T, C), dtype=np.float32)
    for core in range(N_CORES):
        out[core // 4] += results[core]["out"]
    out += np.asarray(b_proj, dtype=np.float32)[None, None, :]
    return out


def kernel(x, w_qkv, w_proj, b_proj):
    nc = _get_compiled()
    res = run_bass_kernel_spmd(nc, _in_maps(x, w_qkv, w_proj),
                               core_ids=list(range(N_CORES)))
    return _combine(res.results, b_proj)


def kernel_traced(x, w_qkv, w_proj, b_proj):
    """Like kernel() but with NTFF tracing; returns (out, BassKernelResults)."""
    nc = _get_compiled()
    res = run_bass_kernel_spmd(nc, _in_maps(x, w_qkv, w_proj),
                               core_ids=list(range(N_CORES)), trace=True)
    return _combine(res.results, b_proj), res
